# revision 1
# baseline (speedup 1.0000x reference)
"""DotInteraction Trainium2 kernel.

features [16384, 27, 128] f32 -> strict-lower-triangle pairwise dots [16384, 351].

Pure data parallel over batch: 2048 samples per core on 8 cores; each core
computes its samples' 27x27 Gram matrices on the PE and ships the blocks
back; the host gathers the tril indices.

Design (evolved v1 151.8us -> 77.8us through trace-driven iterations):

  1. fp16 everywhere off-chip: inputs quantized on host (Gram accumulation
     stays fp32 in PSUM; max rel err 4.9e-4 vs the 2e-2 gate), halving DMA
     bytes and running the PE at 1 cycle/row instead of fp32's 4.
  2. Host pre-transposes and pads: xt [128(D), 512 quartets, 4 samples, 32]
     fp16 (cols 27-31 zero). No on-device transpose. A quartet's 128 cols
     are contiguous -> ONE matmul per quartet (512/core instead of 2048)
     with M=128 contiguous weights, which triggers the compiler's Fast
     Weight Load (2 fp16 weight cols/cycle); sample j's Gram block lands
     32-partition-aligned at partitions 32j (engine APs require 32-aligned
     partition bases). rhs streams only real cols (strided AP, N=108).
  3. Whole-core input resident in one SBUF tile (128KB/partition), filled
     by 33 upfront partial DMAs (512-col first partial so compute starts
     ~2us earlier; 4096B descriptor lines; ~340GB/s observed). No tile
     recycling, no chunk-granularity bubbles.
  4. One PSUM tile spanning all 8 banks [128, 8, 4, 128] f32; round r of 4
     quartets fills bank r%8. Diagonal-block extraction every 2 rounds
     (2 banks): per band j one 4D copy (vector/scalar, 8:7 balanced to
     equalize engine time) moves the 32-aligned 27x26 tril-relevant block
     cols of 8 quartets into gs [128, 512, 26] fp16. 2-bank groups keep 3
     groups of PSUM write-after-read lookahead -- 4-bank groups measured
     ~20us slower from PE stalls on bank recycling.
  5. 8 output DMAs (gpsimd-triggered; sync-triggered measured slower) of
     the packed 3.41MB/core staging buffer.

Per-core budget at 2.4GHz PE / ~340GB/s DMA: input wire ~49us, PE busy
~42us (512 FWL ldweights+matmul pairs), extraction ~50us per copy engine,
plus ~9us fixed boot and ~8us drain/epilogue.
"""
import numpy as np

B, F, D = 16384, 27, 128
NCORES = 8
BL = B // NCORES            # samples per core (2048)
NQ = BL // 4                # quartets per core (512)
NR = NQ // 4                # rounds (128), 4 quartets per round
NG = NR // 2                # extraction groups (64), 8 quartets each
INP = 32                    # input partial DMAs
OUTP = 8                    # output DMAs
SP = NQ * 128               # padded xt cols per core (65536)
CC = F - 1                  # cols kept per Gram block (tril j<=25)
GC = NQ * CC                # gs cols (13312)

_CACHE = {}


def _build():
    import concourse.tile as tile
    from concourse import bacc, mybir

    f16 = mybir.dt.float16
    f32 = mybir.dt.float32
    nc = bacc.Bacc("TRN2", target_bir_lowering=False, debug=False)
    feat = nc.dram_tensor("features", [D, SP], f16, kind="ExternalInput")
    out_d = nc.dram_tensor("out", [128, GC], f16, kind="ExternalOutput")

    # input partial col sizes: small first partial -> compute starts early
    IWS = [512] + [2048] * 31 + [1536]
    assert sum(IWS) == SP
    OW = GC // OUTP         # gs cols per output DMA (1728)

    with tile.TileContext(nc) as tc:
        with (
            tc.tile_pool(name="xt", bufs=1) as xt_pool,
            tc.tile_pool(name="gs", bufs=1) as gs_pool,
            tc.tile_pool(name="pg", bufs=1, space="PSUM") as pg_pool,
        ):
            xt = xt_pool.tile([128, NQ, 4, 32], f16)
            gs = gs_pool.tile([128, NQ, CC], f16)
            xf = xt[:].rearrange("p q s c -> p (q s c)")
            o = 0
            for w in IWS:
                nc.sync.dma_start(xf[:, o:o + w], feat[:, o:o + w])
                o += w

            pg = pg_pool.tile([128, 8, 4, 128], f32)
            for g in range(NG):
                b0 = (2 * g) % 8
                for r in range(2 * g, 2 * g + 2):
                    for s in range(4):
                        Q = 4 * r + s
                        nc.tensor.matmul(
                            pg[:, r % 8, s, 0:4 * F],
                            xt[:, Q, :, :],
                            xt[:, Q, :, 0:F],
                        )
                for j in range(4):
                    src = pg[32 * j:32 * j + F, b0:b0 + 2, :,
                             F * j:F * j + CC]
                    dst = gs[32 * j:32 * j + F, 8 * g:8 * (g + 1), :
                             ].rearrange("p (b s) c -> p b s c", b=2, s=4)
                    if ((4 * g + j) * 8) % 15 < 8:
                        nc.vector.tensor_copy(dst, src)
                    else:
                        nc.scalar.copy(dst, src)

                if g % (NG // OUTP) == NG // OUTP - 1:
                    op = g // (NG // OUTP)
                    nc.gpsimd.dma_start(
                        out_d[:, OW * op:OW * (op + 1)],
                        gs[:].rearrange("p q c -> p (q c)")[
                            :, OW * op:OW * (op + 1)])

    nc.compile()
    return nc


def _run_spmd(nc, in_maps):
    """Like bass2jax.run_bass_via_pjrt multi-core, but builds the global
    sharded arrays from per-device shards (device_put per core) instead of
    one host concat — a single large host->device transfer can fail on the
    axon relay; per-core transfers are fine."""
    import jax
    from jax.experimental.shard_map import shard_map
    from jax.sharding import Mesh, NamedSharding, PartitionSpec
    from concourse import bass2jax, mybir

    bass2jax.install_neuronx_cc_hook()
    partition_name = (nc.partition_id_tensor.name
                      if nc.partition_id_tensor else None)
    in_names, out_names, out_avals = [], [], []
    for alloc in nc.m.functions[0].allocations:
        if not isinstance(alloc, mybir.MemoryLocationSet):
            continue
        name = alloc.memorylocations[0].name
        if alloc.kind == "ExternalInput":
            if name != partition_name:
                in_names.append(name)
        elif alloc.kind == "ExternalOutput":
            out_names.append(name)
            out_avals.append(jax.core.ShapedArray(
                tuple(alloc.tensor_shape), mybir.dt.np(alloc.dtype)))
    n_params = len(in_names)
    n_outs = len(out_names)
    all_in_names = list(in_names) + list(out_names)
    if partition_name is not None:
        all_in_names.append(partition_name)

    def _body(*args):
        operands = list(args)
        if partition_name is not None:
            operands.append(bass2jax.partition_id_tensor())
        outs = bass2jax._bass_exec_p.bind(
            *operands,
            out_avals=tuple(out_avals),
            in_names=tuple(all_in_names),
            out_names=tuple(out_names),
            lowering_input_output_aliases=(),
            sim_require_finite=True,
            sim_require_nnan=True,
            nc=nc,
        )
        return tuple(outs)

    devices = jax.devices()[:NCORES]
    mesh = Mesh(np.asarray(devices), ("core",))
    sharding = NamedSharding(mesh, PartitionSpec("core"))
    donate = tuple(range(n_params, n_params + n_outs))
    sharded = jax.jit(
        shard_map(_body, mesh=mesh,
                  in_specs=(PartitionSpec("core"),) * (n_params + n_outs),
                  out_specs=(PartitionSpec("core"),) * n_outs,
                  check_rep=False),
        donate_argnums=donate, keep_unused=True)

    def _global(per_core):
        shards = [jax.device_put(per_core[c], devices[c])
                  for c in range(NCORES)]
        gshape = (NCORES * per_core[0].shape[0], *per_core[0].shape[1:])
        return jax.make_array_from_single_device_arrays(
            gshape, sharding, shards)

    gins = [_global([np.asarray(m[name]) for m in in_maps])
            for name in in_names]
    gzeros = [_global([np.zeros(av.shape, av.dtype)] * NCORES)
              for av in out_avals]
    out_arrs = sharded(*gins, *gzeros)

    fetched = [np.asarray(a).reshape(NCORES, *out_avals[i].shape)
               for i, a in enumerate(out_arrs)]
    return [{name: fetched[i][c] for i, name in enumerate(out_names)}
            for c in range(NCORES)]


def kernel(features: np.ndarray) -> np.ndarray:
    features = np.asarray(features, dtype=np.float32)
    assert features.shape == (B, F, D), features.shape

    if "nc" not in _CACHE:
        _CACHE["nc"] = _build()
    nc = _CACHE["nc"]

    # [B, F, D] -> fp16 -> per-core padded X^T [D, NQ, 4, 32]
    f16 = features.astype(np.float16)
    xp = np.zeros((NCORES, D, NQ, 4, 32), dtype=np.float16)
    xp[..., :F] = f16.reshape(NCORES, NQ, 4, F, D).transpose(0, 4, 1, 2, 3)
    xp = xp.reshape(NCORES, D, SP)
    in_maps = [{"features": xp[c]} for c in range(NCORES)]

    results = _run_spmd(nc, in_maps)

    # [NCORES][128, 13312] fp16: [32j+r, 26Q + c] = G_{4Q+j}[r, c], c<26
    dump = np.stack([r["out"] for r in results])          # [8, 128, 13312]
    v = dump.reshape(NCORES, 128, NQ, CC)                 # [c, p, Q, col]
    G = np.empty((NCORES, NQ, 4, F, CC), dtype=np.float16)
    for j in range(4):
        G[:, :, j] = v[:, 32 * j:32 * j + F].transpose(0, 2, 1, 3)
    G = G.reshape(B, F, CC)

    rows, cols = np.tril_indices(F, k=-1)
    return G[:, rows, cols].astype(np.float32)

